# revision 10
# baseline (speedup 1.0000x reference)
"""Causal multi-head attention on 8 Trainium2 NeuronCores.

Problem: x [4, 2048, 1024], 16 heads x dk=64, causal attention + output proj.

Sharding: 8 cores = 4 batches x 2 head-groups (8 heads each).
Each core computes, for its (batch b, head-group g), all in bf16:
    qT/kT = Wq_g x_b^T           [512, 2048]  ([head*dk, seq])
    v     = x_b Wv_g^T           [2048, 512]  ([seq, head*dk])
    per head-pair hp (q-tile 512 wide, j-tile 128 wide, causal):
        sT pair = kT_h^T-block @ qT_h   [j 128, q 512]  (row-packed K=64 pair)
        pp  = exp(sT/8) * causal_mask   (ACT exp -> bf16)
        po[0:64]  += v_a^T @ pp_a  \  col-packed concurrent pair
        po[64:128]+= v_b^T @ pp_b  /  (tile_position (0,0) / (0,64))
        S_acc += pp                     (DVE, bf16 -> softmax denominator)
      strip end: Z = ones^T @ S_acc (2 concurrent MMs), recip on DVE,
        broadcast 1/Z via K=1 selector matmuls, ohT = po * bcr (DVE).
    yT_partial = Wo_g^T-slice @ ohT    [1024, 2048]
Host: y_b = (yT_{b,0} + yT_{b,1})^T.

Schedule: st=0 projections up front; projections for st=qt+1 and the
output projections of earlier q-tiles are dripped between attention
blocks so the PE never idles and the ACT-engine exp is hidden.
"""

import sys

if "/opt/trn_rl_repo" not in sys.path:
    sys.path.insert(0, "/opt/trn_rl_repo")

import numpy as np

import concourse.bass as bass
import concourse.mybir as mybir
from concourse import bacc, tile
from concourse.bass_utils import run_bass_kernel_spmd

P = 128
D_MODEL = 1024
NUM_HEADS = 16
DK = 64
B, S = 4, 2048
HG = NUM_HEADS // 2  # 8 heads per group
MG = HG * DK  # 512 columns per head-group
N_CORES = 8

QT = S // 512  # 4 q-tiles of 512
JT = S // P  # 16 j-tiles of 128
KT = D_MODEL // P  # 8 contraction tiles for projections
MSUB = MG // P  # 4 m-subtiles (head pairs)
NT = D_MODEL // P  # 8 output-proj n-tiles

F32 = mybir.dt.float32
F32R = mybir.dt.float32r
BF16 = mybir.dt.bfloat16
EXP = mybir.ActivationFunctionType.Exp
LN = mybir.ActivationFunctionType.Ln
MULT = mybir.AluOpType.mult
ADD = mybir.AluOpType.add

_CACHED_NC = None


def build_nc() -> bass.Bass:
    nc = bacc.Bacc("TRN2", target_bir_lowering=False, debug=False)

    xT = nc.dram_tensor("xT", [D_MODEL, S], BF16, kind="ExternalInput")
    wqT = nc.dram_tensor("wqT", [D_MODEL, MG], BF16, kind="ExternalInput")
    wkT = nc.dram_tensor("wkT", [D_MODEL, MG], BF16, kind="ExternalInput")
    wvT = nc.dram_tensor("wvT", [D_MODEL, MG], BF16, kind="ExternalInput")
    woT = nc.dram_tensor("woT", [MG, D_MODEL], BF16, kind="ExternalInput")
    masks = nc.dram_tensor("masks", [P, P], BF16, kind="ExternalInput")
    yT = nc.dram_tensor("yT", [D_MODEL, S], F32, kind="ExternalOutput")

    xT_t = xT.rearrange("(kt p) s -> p kt s", p=P)
    wq_t = wqT.rearrange("(kt p) m -> p kt m", p=P)
    wk_t = wkT.rearrange("(kt p) m -> p kt m", p=P)
    wv_t = wvT.rearrange("(kt p) m -> p kt m", p=P)
    wo_t = woT.rearrange("(kt p) n -> p kt n", p=P)
    yT_t = yT.rearrange("(nt p) s -> p nt s", p=P)

    with tile.TileContext(nc) as tc:
        with (
            tc.tile_pool(name="wpool", bufs=1) as wpool,
            tc.tile_pool(name="qkv", bufs=1) as qkv,
            tc.tile_pool(name="wqkv", bufs=1) as wqkv,
            tc.tile_pool(name="xs", bufs=2) as xs,
            tc.tile_pool(name="sacc", bufs=2) as saccp,
            tc.tile_pool(name="oh", bufs=4) as ohp,
            tc.tile_pool(name="attn", bufs=3) as attn,
            tc.tile_pool(name="attnc", bufs=1) as attnc,
            tc.tile_pool(name="ys", bufs=4) as ysp,
            tc.tile_pool(name="ps_s", bufs=2, space="PSUM") as ps_s,
            tc.tile_pool(name="ps_o", bufs=1, space="PSUM") as ps_o,
            tc.tile_pool(name="ps_m", bufs=2, space="PSUM") as ps_m,
        ):
            # ---- static tiles ----
            wo_sb = wpool.tile([P, MSUB, D_MODEL], BF16, tag="wo")
            qT_sb = qkv.tile([P, MSUB, S], BF16, tag="qT")
            kT_sb = qkv.tile([P, MSUB, S], BF16, tag="kT")
            v_sb = qkv.tile([P, JT, HG, DK], BF16, tag="v")
            mask2 = attnc.tile([P, 2, P], BF16, tag="mask2")
            ones_sb = attnc.tile([P, 1], BF16, tag="ones")
            zln = attnc.tile([33, 512], F32, tag="zln")
            zq = attnc.tile([33, 512], BF16, tag="zq")
            bcd = attnc.tile([P, 512], BF16, tag="bcd")
            # selectors on partitions 0 / 32 (matching zq rows):
            # sel_a lights out-partitions 0:64, sel_b lights 64:128
            sel2 = attnc.tile([33, P], BF16, tag="sel2")
            nc.vector.memset(sel2[0:1, :], 0.0)
            nc.vector.memset(sel2[32:33, :], 0.0)
            nc.vector.memset(sel2[0:1, 0:DK], 1.0)
            nc.vector.memset(sel2[32:33, DK:P], 1.0)
            nc.vector.memset(ones_sb[:], 1.0)
            nc.sync.dma_start(mask2[:, 0, :], masks[:])
            nc.sync.dma_start(mask2[:, 1, :], masks[:])

            # ---- input DMAs (x0+wq interleaved so first matmuls start early)
            w_sb = {}
            for name in ("q", "k", "v"):
                w_sb[name] = wqkv.tile([P, KT, MG], BF16, tag=f"w{name}", name=f"w{name}")
            x_tiles = {}
            for st in (0, 1):
                x_tiles[st] = xs.tile([P, KT, 512], BF16, tag="x", name=f"x{st}")
            for kt in range(KT):
                nc.sync.dma_start(x_tiles[0][:, kt], xT_t[:, kt, 0:512])
                nc.sync.dma_start(w_sb["q"][:, kt], wq_t[:, kt])
            for name, wsrc in (("k", wk_t), ("v", wv_t)):
                for kt in range(KT):
                    nc.sync.dma_start(w_sb[name][:, kt], wsrc[:, kt])
            for kt in range(MSUB):
                nc.sync.dma_start(wo_sb[:, kt], wo_t[:, kt])
            for kt in range(KT):
                nc.sync.dma_start(x_tiles[1][:, kt], xT_t[:, kt, 512:1024])

            # ---- projection group emitters ----
            def emit_proj_qk(name, dst, mt, st):
                x_t = x_tiles[st]
                ssl = slice(st * 512, (st + 1) * 512)
                w = w_sb[name]
                msl = slice(mt * P, (mt + 1) * P)
                pt = ps_m.tile([P, 512], F32, tag="ms", name=f"p{name}{st}{mt}")
                for kt in range(KT):
                    nc.tensor.matmul(
                        pt[:], w[:, kt, msl], x_t[:, kt],
                        start=(kt == 0), stop=(kt == KT - 1),
                    )
                if st < 2:
                    nc.scalar.copy(dst[:, mt, ssl], pt[:])
                else:
                    nc.vector.tensor_copy(dst[:, mt, ssl], pt[:])

            def emit_proj_v(ssub, st):
                x_t = x_tiles[st]
                jt_ = st * 4 + ssub
                s0 = ssub * P
                pt = ps_m.tile([P, 512], F32, tag="ms", name=f"pv{st}{ssub}")
                for kt in range(KT):
                    nc.tensor.matmul(
                        pt[:], x_t[:, kt, s0 : s0 + P], w_sb["v"][:, kt],
                        start=(kt == 0), stop=(kt == KT - 1),
                    )
                if st < 2:
                    nc.scalar.copy(
                        v_sb[:, jt_, :, :], pt.rearrange("p (h d) -> p h d", h=HG)
                    )
                else:
                    nc.vector.tensor_copy(
                        v_sb[:, jt_, :, :], pt.rearrange("p (h d) -> p h d", h=HG)
                    )

            def proj_items(st):
                items = []
                for name, dst in (("q", qT_sb), ("k", kT_sb)):
                    for mt in range(MSUB):
                        items.append(lambda n=name, d=dst, m=mt: emit_proj_qk(n, d, m, st))
                for ssub in range(4):
                    items.append(lambda s=ssub: emit_proj_v(s, st))
                return items

            def emit_outproj(ohT_prev, qt_prev, nt, y_eng="v"):
                qsl_p = slice(qt_prev * 512, (qt_prev + 1) * 512)
                py = ps_m.tile([P, 512], F32, tag="ms", name=f"py{qt_prev}{nt}")
                for mt in range(MSUB):
                    nc.tensor.matmul(
                        py[:],
                        wo_sb[:, mt, nt * P : (nt + 1) * P],
                        ohT_prev[:, mt, :],
                        start=(mt == 0),
                        stop=(mt == MSUB - 1),
                    )
                y_sb = ysp.tile([P, 512], F32, tag="y")
                if y_eng == "s":
                    nc.scalar.copy(y_sb[:], py[:])
                else:
                    nc.vector.tensor_copy(y_sb[:], py[:])
                nc.sync.dma_start(yT_t[:, nt, qsl_p], y_sb[:])

            def outproj_items(ohT_prev, qt_prev, y_eng="v"):
                return [
                    lambda n=nt: emit_outproj(ohT_prev, qt_prev, n, y_eng)
                    for nt in range(NT)
                ]

            def emit_scores(qt, hp, jt):
                """scores^T [j, q] pair for head pair hp, row-packed K=64."""
                jsl = slice(jt * P, (jt + 1) * P)
                di = jt - qt * 4
                delta = 128 * di if di >= 0 else 0
                qsl_d = slice(qt * 512 + delta, (qt + 1) * 512)
                ss = ps_s.tile([P, 2, 512], F32, tag="ss")
                nc.tensor.matmul(
                    ss[:, 0, delta:],
                    kT_sb[0:DK, hp, jsl],
                    qT_sb[0:DK, hp, qsl_d],
                    start=True, stop=True,
                )
                nc.tensor.matmul(
                    ss[:, 1, delta:],
                    kT_sb[DK:P, hp, jsl],
                    qT_sb[DK:P, hp, qsl_d],
                    start=True, stop=True,
                )
                return ss, delta

            # ---- phase A: st=0 projections ----
            for it in proj_items(0):
                it()

            # ---- main loop: attention with dripped proj/outproj ----
            oh_tiles = {}
            for qt in range(QT):
                njt = 4 * (qt + 1)
                work = [(hp, jt) for hp in range(MSUB) for jt in range(njt)]
                ohT = ohp.tile([P, MSUB, 512], BF16, tag="ohT")
                oh_tiles[qt] = ohT

                # x prefetch for the tile dripped NEXT qt
                if qt + 2 <= 3 and (qt + 2) not in x_tiles:
                    x_tiles[qt + 2] = xs.tile([P, KT, 512], BF16, tag="x", name=f"x{qt+2}")
                    ssl = slice((qt + 2) * 512, (qt + 3) * 512)
                    for kt in range(KT):
                        nc.sync.dma_start(x_tiles[qt + 2][:, kt], xT_t[:, kt, ssl])

                # drip: proj for st=qt+1; all earlier output projections
                # reserved for the ACT-heavy final q-tile
                drip = []
                if qt < 3:
                    drip += proj_items(qt + 1)
                if qt == 3:
                    for qp in (0, 1, 2):
                        drip += outproj_items(oh_tiles[qp], qp)

                # weighted drip: extra PE filler right after each strip
                # start so the strip-end Z chain latency is hidden
                weights = [3 if jt < 2 else 1 for (_hp, jt) in work]
                wtot = sum(weights)
                wacc = 0.0
                dripped = 0
                sacc_t = None
                po_a = po_b = None
                pend = emit_scores(qt, 0, 0)
                for wi, (hp, jt) in enumerate(work):
                    if jt == 0:
                        po_a = ps_o.tile([P, 512], F32, tag="poA", name="po_a")
                        po_b = ps_o.tile([P, 512], F32, tag="poB", name="po_b")
                    ss, delta = pend
                    di = jt - qt * 4
                    pp = attn.tile([P, 2, 512], BF16, tag="pp")
                    nc.scalar.activation(
                        pp[:, :, delta:], ss[:, :, delta:], EXP, scale=0.125
                    )
                    if di >= 0:  # mask the 128-wide staircase window
                        wsl = slice(delta, delta + P)
                        nc.gpsimd.tensor_tensor(
                            pp[:, :, wsl], pp[:, :, wsl], mask2[:], MULT
                        )
                    # softmax denominator partial sums (per j-slot)
                    if jt == 0:
                        sacc_t = saccp.tile([P, 2, 512], BF16, tag="sa")
                        nc.vector.tensor_copy(sacc_t[:], pp[:])
                    else:
                        nc.vector.tensor_tensor(
                            sacc_t[:, :, delta:],
                            sacc_t[:, :, delta:],
                            pp[:, :, delta:],
                            ADD,
                        )
                    # issue next block's scores before this block's attn@V
                    if wi + 1 < len(work):
                        pend = emit_scores(qt, *work[wi + 1])
                    # drip PE-filler work (projections / output proj)
                    wacc += weights[wi] * len(drip) / wtot
                    while dripped < int(wacc) and dripped < len(drip):
                        drip[dripped]()
                        dripped += 1
                    # attn @ V: col-packed concurrent pair (0,0)/(0,64)
                    nc.tensor.matmul(
                        po_a[0:DK, delta:],
                        v_sb[:, jt, 2 * hp, :],
                        pp[:, 0, delta:],
                        start=(jt == 0),
                        stop=(jt == njt - 1),
                    )
                    nc.tensor.matmul(
                        po_b[DK:P, delta:],
                        v_sb[:, jt, 2 * hp + 1, :],
                        pp[:, 1, delta:],
                        start=(jt == 0),
                        stop=(jt == njt - 1),
                    )
                    if jt == njt - 1:
                        # softmax denominators: two concurrent K=128 M=1
                        # matmuls into col groups 0 / 32 of one psum bank
                        # (data is safe: no further accumulation there)
                        z2 = ps_m.tile([P, 512], F32, tag="ms", name="z2")
                        nc.tensor.matmul(
                            z2[0:1, :], ones_sb[:], sacc_t[:, 0, :],
                            start=True, stop=True,
                        )
                        nc.tensor.matmul(
                            z2[32:33, :], ones_sb[:], sacc_t[:, 1, :],
                            start=True, stop=True,
                        )
                        # 1/Z = exp(-ln Z) on ACT (ln+exp share one table)
                        nc.scalar.activation(zln[0:1, :], z2[0:1, :], LN)
                        nc.scalar.activation(zln[32:33, :], z2[32:33, :], LN)
                        nc.scalar.activation(zq[0:1, :], zln[0:1, :], EXP, scale=-1.0)
                        nc.scalar.activation(
                            zq[32:33, :], zln[32:33, :], EXP, scale=-1.0
                        )
                        # broadcast 1/Z to all partitions via selector MMs
                        bcr = ps_m.tile([P, 512], F32, tag="ms", name="bcr")
                        nc.tensor.matmul(
                            bcr[:], sel2[0:1, :], zq[0:1, :],
                            start=True, stop=False,
                        )
                        nc.tensor.matmul(
                            bcr[:], sel2[32:33, :], zq[32:33, :],
                            start=False, stop=True,
                        )
                        nc.vector.tensor_copy(bcd[:], bcr[:])
                        nc.vector.tensor_tensor(
                            ohT[0:DK, hp, :], po_a[0:DK, :], bcd[0:DK, :], MULT
                        )
                        nc.vector.tensor_tensor(
                            ohT[DK:P, hp, :], po_b[DK:P, :], bcd[DK:P, :], MULT
                        )
                while dripped < len(drip):
                    drip[dripped]()
                    dripped += 1
            # final q-tile's output projection
            for i, it in enumerate(outproj_items(oh_tiles[3], 3, y_eng="s")):
                it()

    nc.finalize()
    return nc


def _get_nc() -> bass.Bass:
    global _CACHED_NC
    if _CACHED_NC is None:
        _CACHED_NC = build_nc()
    return _CACHED_NC


def _make_masks() -> np.ndarray:
    j = np.arange(P)[:, None]
    w = np.arange(P)[None, :]
    import ml_dtypes

    return (w >= j).astype(ml_dtypes.bfloat16)


def make_in_maps(inputs):
    import ml_dtypes

    bf16 = ml_dtypes.bfloat16
    x = np.asarray(inputs["x"], dtype=np.float32)
    q_heads = np.asarray(inputs["q_heads"], dtype=np.float32)
    k_heads = np.asarray(inputs["k_heads"], dtype=np.float32)
    v_heads = np.asarray(inputs["v_heads"], dtype=np.float32)
    output_proj = np.asarray(inputs["output_proj"], dtype=np.float32)

    masks = _make_masks()
    in_maps = []
    for core in range(N_CORES):
        b, g = divmod(core, 2)
        gsl = slice(g * MG, (g + 1) * MG)
        in_maps.append(
            {
                "xT": np.ascontiguousarray(x[b].T).astype(bf16),
                "wqT": np.ascontiguousarray(q_heads[gsl].T).astype(bf16),
                "wkT": np.ascontiguousarray(k_heads[gsl].T).astype(bf16),
                "wvT": np.ascontiguousarray(v_heads[gsl].T).astype(bf16),
                "woT": np.ascontiguousarray(output_proj[:, gsl].T).astype(bf16),
                "masks": masks,
            }
        )
    return in_maps


def kernel(x, q_heads, k_heads, v_heads, output_proj):
    in_maps = make_in_maps(
        {
            "x": x,
            "q_heads": q_heads,
            "k_heads": k_heads,
            "v_heads": v_heads,
            "output_proj": output_proj,
        }
    )
    nc = _get_nc()
    res = run_bass_kernel_spmd(nc, in_maps, list(range(N_CORES)))
    y = np.empty((B, S, D_MODEL), np.float32)
    for b in range(B):
        acc = res.results[2 * b]["yT"] + res.results[2 * b + 1]["yT"]
        y[b] = acc.T
    return y


# revision 14
# speedup vs baseline: 1.3568x; 1.3568x over previous
"""Causal MHA on 8 TRN2 cores — 4 batches x 2 head-groups, bf16 compute.

Per core (batch b, head-group g):
    qT/kT = Wq_g x_b^T [512,2048], v = x_b Wv_g^T [2048,512] (bf16 matmuls)
    attention per head-pair strip (q-tile 512, j-tile 128, causal):
        sT pair = kT^T-block @ qT (row-packed K=64 concurrent pair)
        pp = exp(sT/8) * mask  (ACT -> bf16)
        PV[65,q] += v_ext^T @ pp  (ones column -> Z in row 64)
        ohT = PV[0:64] * bcast(1/Z)  (selector MMs + DVE recip)
    yT_partial = Wo_g^T-slice @ ohT (bf16)
Host: y_b = (yT_{b,0} + yT_{b,1})^T.

Schedule: st=0 projections up front; projections for st=qt+1 drip into
qt's attention blocks (keeps PE dense/warm and hides the ACT exp);
output projections of q0..q2 are reserved for the ACT-heavy qt2/qt3.
"""

import sys

if "/opt/trn_rl_repo" not in sys.path:
    sys.path.insert(0, "/opt/trn_rl_repo")

import numpy as np

import concourse.bass as bass
import concourse.mybir as mybir
from concourse import bacc, tile
from concourse.bass_utils import run_bass_kernel_spmd

P = 128
D_MODEL = 1024
NUM_HEADS = 16
DK = 64
B, S = 4, 2048
HG = NUM_HEADS // 2
MG = HG * DK
N_CORES = 8

QT = S // 512
JT = S // P
KT = D_MODEL // P
MSUB = MG // P
NT = D_MODEL // P

F32 = mybir.dt.float32
F32R = mybir.dt.float32r
BF16 = mybir.dt.bfloat16
EXP = mybir.ActivationFunctionType.Exp
MULT = mybir.AluOpType.mult

_CACHED_NC = None


def build_nc() -> bass.Bass:
    nc = bacc.Bacc("TRN2", target_bir_lowering=False, debug=False)

    xT = nc.dram_tensor("xT", [D_MODEL, S], BF16, kind="ExternalInput")
    wqT = nc.dram_tensor("wqT", [D_MODEL, MG], BF16, kind="ExternalInput")
    wkT = nc.dram_tensor("wkT", [D_MODEL, MG], BF16, kind="ExternalInput")
    wvT = nc.dram_tensor("wvT", [D_MODEL, MG], BF16, kind="ExternalInput")
    woT = nc.dram_tensor("woT", [MG, D_MODEL], BF16, kind="ExternalInput")
    masks = nc.dram_tensor("masks", [P, P], BF16, kind="ExternalInput")
    yT = nc.dram_tensor("yT", [D_MODEL, S], F32, kind="ExternalOutput")

    xT_t = xT.rearrange("(kt p) s -> p kt s", p=P)
    wq_t = wqT.rearrange("(kt p) m -> p kt m", p=P)
    wk_t = wkT.rearrange("(kt p) m -> p kt m", p=P)
    wv_t = wvT.rearrange("(kt p) m -> p kt m", p=P)
    wo_t = woT.rearrange("(kt p) n -> p kt n", p=P)
    yT_t = yT.rearrange("(nt p) s -> p nt s", p=P)

    with tile.TileContext(nc) as tc:
        with (
            tc.tile_pool(name="wpool", bufs=1) as wpool,
            tc.tile_pool(name="qkv", bufs=1) as qkv,
            tc.tile_pool(name="wqkv", bufs=1) as wqkv,
            tc.tile_pool(name="xs", bufs=2) as xs,
            tc.tile_pool(name="oh", bufs=4) as ohp,
            tc.tile_pool(name="attn", bufs=3) as attn,
            tc.tile_pool(name="attnc", bufs=1) as attnc,
            tc.tile_pool(name="ys", bufs=4) as ysp,
            tc.tile_pool(name="ps_s", bufs=2, space="PSUM") as ps_s,
            tc.tile_pool(name="ps_o", bufs=1, space="PSUM") as ps_o,
            tc.tile_pool(name="ps_y", bufs=1, space="PSUM") as ps_y,
        ):
            # ---- static tiles ----
            wo_sb = wpool.tile([P, MSUB, D_MODEL], BF16, tag="wo")
            qT_sb = qkv.tile([P, MSUB, S], BF16, tag="qT")
            kT_sb = qkv.tile([P, MSUB, S], BF16, tag="kT")
            # v with ones column: [j-part, jt, head, dk+1] (col 64 -> Z)
            v_sb = qkv.tile([P, JT, HG, DK + 1], BF16, tag="v")
            nc.vector.memset(v_sb[:, :, :, DK : DK + 1], 1.0)
            mask2 = attnc.tile([P, 2, P], BF16, tag="mask2")
            sel_a = attnc.tile([1, P], F32R, tag="sel_a")
            sel_b = attnc.tile([1, P], F32R, tag="sel_b")
            nc.vector.memset(sel_a[:].bitcast(F32), 0.0)
            nc.vector.memset(sel_b[:].bitcast(F32), 0.0)
            nc.vector.memset(sel_a[0:1, 0:DK].bitcast(F32), 1.0)
            nc.vector.memset(sel_b[0:1, DK:P].bitcast(F32), 1.0)
            nc.sync.dma_start(mask2[:, 0, :], masks[:])
            nc.sync.dma_start(mask2[:, 1, :], masks[:])

            # ---- input DMAs: x0 + wq interleaved so matmuls start early
            w_sb = {}
            for name in ("q", "k", "v"):
                w_sb[name] = wqkv.tile([P, KT, MG], BF16, tag=f"w{name}", name=f"w{name}")
            x_tiles = {}
            for st in (0, 1):
                x_tiles[st] = xs.tile([P, KT, 512], BF16, tag="x", name=f"x{st}")
            for kt in range(KT):
                nc.sync.dma_start(x_tiles[0][:, kt], xT_t[:, kt, 0:512])
                nc.sync.dma_start(w_sb["q"][:, kt], wq_t[:, kt])
            for name, wsrc in (("k", wk_t), ("v", wv_t)):
                for kt in range(KT):
                    nc.sync.dma_start(w_sb[name][:, kt], wsrc[:, kt])
            for kt in range(MSUB):
                nc.sync.dma_start(wo_sb[:, kt], wo_t[:, kt])
            for kt in range(KT):
                nc.sync.dma_start(x_tiles[1][:, kt], xT_t[:, kt, 512:1024])

            # ---- projection groups (pt shares the ps_y "py" ring) ----
            def emit_proj_qk(name, dst, mt, st):
                x_t = x_tiles[st]
                ssl = slice(st * 512, (st + 1) * 512)
                msl = slice(mt * P, (mt + 1) * P)
                pt = ps_y.tile([P, 512], F32, tag="py", name=f"p{name}{st}{mt}")
                for kt in range(KT):
                    nc.tensor.matmul(
                        pt[:], w_sb[name][:, kt, msl], x_t[:, kt],
                        start=(kt == 0), stop=(kt == KT - 1),
                    )
                nc.vector.tensor_copy(dst[:, mt, ssl], pt[:])

            def emit_proj_v(ssub, st):
                x_t = x_tiles[st]
                jt_ = st * 4 + ssub
                s0 = ssub * P
                pt = ps_y.tile([P, 512], F32, tag="py", name=f"pv{st}{ssub}")
                for kt in range(KT):
                    nc.tensor.matmul(
                        pt[:], x_t[:, kt, s0 : s0 + P], w_sb["v"][:, kt],
                        start=(kt == 0), stop=(kt == KT - 1),
                    )
                nc.vector.tensor_copy(
                    v_sb[:, jt_, :, 0:DK], pt.rearrange("p (h d) -> p h d", h=HG)
                )

            def proj_items(st):
                items = []
                for name, dst in (("q", qT_sb), ("k", kT_sb)):
                    for mt in range(MSUB):
                        items.append(lambda n=name, d=dst, m=mt: emit_proj_qk(n, d, m, st))
                for ssub in range(4):
                    items.append(lambda s=ssub: emit_proj_v(s, st))
                return items

            def emit_outproj(ohT_prev, qt_prev, nt):
                qsl_p = slice(qt_prev * 512, (qt_prev + 1) * 512)
                py = ps_y.tile([P, 512], F32, tag="py", name=f"py{qt_prev}{nt}")
                for mt in range(MSUB):
                    nc.tensor.matmul(
                        py[:],
                        wo_sb[:, mt, nt * P : (nt + 1) * P],
                        ohT_prev[:, mt, :],
                        start=(mt == 0),
                        stop=(mt == MSUB - 1),
                    )
                y_sb = ysp.tile([P, 512], F32, tag="y")
                if nt % 2 == 0:
                    nc.vector.tensor_copy(y_sb[:], py[:])
                else:
                    nc.scalar.copy(y_sb[:], py[:])
                nc.sync.dma_start(yT_t[:, nt, qsl_p], y_sb[:])

            def outproj_items(ohT_prev, qt_prev):
                return [
                    lambda n=nt: emit_outproj(ohT_prev, qt_prev, n) for nt in range(NT)
                ]

            def emit_scores(qt, hp, jt):
                jsl = slice(jt * P, (jt + 1) * P)
                di = jt - qt * 4
                delta = 128 * di if di >= 0 else 0
                qsl_d = slice(qt * 512 + delta, (qt + 1) * 512)
                ss = ps_s.tile([P, 2, 512], F32, tag="ss")
                nc.tensor.matmul(
                    ss[:, 0, delta:],
                    kT_sb[0:DK, hp, jsl],
                    qT_sb[0:DK, hp, qsl_d],
                    start=True, stop=True,
                )
                nc.tensor.matmul(
                    ss[:, 1, delta:],
                    kT_sb[DK:P, hp, jsl],
                    qT_sb[DK:P, hp, qsl_d],
                    start=True, stop=True,
                )
                return ss, delta

            # ---- phase A: st=0 projections ----
            for it in proj_items(0):
                it()

            # ---- attention with dripped proj / outproj ----
            oh_tiles = {}
            for qt in range(QT):
                njt = 4 * (qt + 1)
                work = [(hp, jt) for hp in range(MSUB) for jt in range(njt)]
                ohT = ohp.tile([P, MSUB, 512], BF16, tag="ohT")
                oh_tiles[qt] = ohT

                if qt + 2 <= 3 and (qt + 2) not in x_tiles:
                    x_tiles[qt + 2] = xs.tile([P, KT, 512], BF16, tag="x", name=f"x{qt+2}")
                    ssl = slice((qt + 2) * 512, (qt + 3) * 512)
                    for kt in range(KT):
                        nc.sync.dma_start(x_tiles[qt + 2][:, kt], xT_t[:, kt, ssl])

                drip = []
                if qt < 3:
                    drip += proj_items(qt + 1)
                if qt == 2:
                    drip += outproj_items(oh_tiles[0], 0)
                if qt == 3:
                    drip += outproj_items(oh_tiles[1], 1)
                    drip += outproj_items(oh_tiles[2], 2)

                per = len(drip) / len(work)
                dripped = 0
                po = {}
                pend = emit_scores(qt, 0, 0)
                for wi, (hp, jt) in enumerate(work):
                    if jt == 0:
                        po[hp] = (
                            ps_o.tile([DK + 1, 512], F32, tag="poA", name="po_a"),
                            ps_o.tile([DK + 1, 512], F32, tag="poB", name="po_b"),
                        )
                    ss, delta = pend
                    di = jt - qt * 4
                    pp = attn.tile([P, 2, 512], BF16, tag="pp")
                    nc.scalar.activation(
                        pp[:, :, delta:], ss[:, :, delta:], EXP, scale=0.125
                    )
                    if di >= 0:
                        wsl = slice(delta, delta + P)
                        nc.vector.tensor_tensor(
                            pp[:, :, wsl], pp[:, :, wsl], mask2[:], MULT
                        )
                    # issue next block's scores before this block's attn@V
                    if wi + 1 < len(work):
                        pend = emit_scores(qt, *work[wi + 1])
                    # drip PE-filler (projections / output projections)
                    while dripped < int(per * (wi + 1)) and dripped < len(drip):
                        drip[dripped]()
                        dripped += 1
                    po_a, po_b = po[hp]
                    nc.tensor.matmul(
                        po_a[:, delta:],
                        v_sb[:, jt, 2 * hp, :],
                        pp[:, 0, delta:],
                        start=(jt == 0),
                        stop=(jt == njt - 1),
                    )
                    nc.tensor.matmul(
                        po_b[:, delta:],
                        v_sb[:, jt, 2 * hp + 1, :],
                        pp[:, 1, delta:],
                        start=(jt == 0),
                        stop=(jt == njt - 1),
                    )
                    if jt == njt - 1:
                        # Z sits in row 64 of each po; broadcast 1/Z via
                        # accumulating selector matmuls, then one recip+mult
                        z2 = attnc.tile([1, 1024], F32R, tag="z2")
                        nc.vector.tensor_copy(z2[0:1, 0:512], po_a[DK : DK + 1, :])
                        nc.vector.tensor_copy(z2[0:1, 512:], po_b[DK : DK + 1, :])
                        dst = ohT[:, hp, :]
                        nc.scalar.copy(dst[0:DK], po_a[0:DK, :])
                        nc.vector.tensor_copy(dst[DK:P], po_b[0:DK, :])
                        bcz = ps_y.tile([P, 512], F32, tag="bcz", name="bcz")
                        nc.tensor.matmul(
                            bcz[:], sel_a[:], z2[0:1, 0:512], start=True, stop=False
                        )
                        nc.tensor.matmul(
                            bcz[:], sel_b[:], z2[0:1, 512:], start=False, stop=True
                        )
                        bcr = attnc.tile([P, 512], F32, tag="bcr")
                        with nc.allow_low_precision(reason="softmax recip"):
                            nc.vector.reciprocal(bcr[:], bcz[:])
                        nc.vector.tensor_tensor(dst, dst, bcr[:], MULT)
                while dripped < len(drip):
                    drip[dripped]()
                    dripped += 1
            # final q-tile's output projection
            for it in outproj_items(oh_tiles[3], 3):
                it()

    nc.finalize()
    return nc


def _get_nc() -> bass.Bass:
    global _CACHED_NC
    if _CACHED_NC is None:
        _CACHED_NC = build_nc()
    return _CACHED_NC


def _make_masks() -> np.ndarray:
    j = np.arange(P)[:, None]
    w = np.arange(P)[None, :]
    import ml_dtypes

    return (w >= j).astype(ml_dtypes.bfloat16)


def make_in_maps(inputs):
    import ml_dtypes

    bf16 = ml_dtypes.bfloat16
    x = np.asarray(inputs["x"], dtype=np.float32)
    q_heads = np.asarray(inputs["q_heads"], dtype=np.float32)
    k_heads = np.asarray(inputs["k_heads"], dtype=np.float32)
    v_heads = np.asarray(inputs["v_heads"], dtype=np.float32)
    output_proj = np.asarray(inputs["output_proj"], dtype=np.float32)

    masks = _make_masks()
    in_maps = []
    for core in range(N_CORES):
        b, g = divmod(core, 2)
        gsl = slice(g * MG, (g + 1) * MG)
        in_maps.append(
            {
                "xT": np.ascontiguousarray(x[b].T).astype(bf16),
                "wqT": np.ascontiguousarray(q_heads[gsl].T).astype(bf16),
                "wkT": np.ascontiguousarray(k_heads[gsl].T).astype(bf16),
                "wvT": np.ascontiguousarray(v_heads[gsl].T).astype(bf16),
                "woT": np.ascontiguousarray(output_proj[:, gsl].T).astype(bf16),
                "masks": masks,
            }
        )
    return in_maps


def kernel(x, q_heads, k_heads, v_heads, output_proj):
    in_maps = make_in_maps(
        {
            "x": x,
            "q_heads": q_heads,
            "k_heads": k_heads,
            "v_heads": v_heads,
            "output_proj": output_proj,
        }
    )
    nc = _get_nc()
    res = run_bass_kernel_spmd(nc, in_maps, list(range(N_CORES)))
    y = np.empty((B, S, D_MODEL), np.float32)
    for b in range(B):
        acc = res.results[2 * b]["yT"] + res.results[2 * b + 1]["yT"]
        y[b] = acc.T
    return y
